# revision 41
# baseline (speedup 1.0000x reference)
"""Trainium2 Bass kernel for nn_Gate_Net (Toeplitz + hard-sigmoid prob + cumprod gate).

Reference computation (per document row of 1024 scores):
  s = doc[1:-1]                      # n = 1022
  score_hat[r, j] = s[j-1-r] if j-1-r >= 0 else 0      # [1021, 1022]
  p[r, j] = clamp(10*(score_hat - s[j]) + 1, 0, 1)      # hard branch, res=0.1
  fwd = cumprod(p, axis=0); bwd = same with s reversed
  out = stack([fwd, bwd]) per doc -> full [32, 2, 1021, 1022] f32

Device algorithm (per doc-dir, column-major, transpose-free):
  Column j's factor sequence over rows m is
    q(j, m) = clamp(g[j-1-m] + c_j, 0, 1),  g[x] = 10*s[x] (0 outside [0,n)),
    c_j = 1 - 10*s[j].
  For m >= j the factor is the constant qt_j = clamp(c_j, 0, 1), so each
  column is geometric below its diagonal:
    out[m, j] = R_j * qt_j^(m-W+1)  for m >= W = 128*(jb+1) >= j+1,
  with R_j the cumprod at row W-1.  One band DMA per doc-dir materializes
  Ball[p, t] = arrR[p + t] (arrR = [0, reversed(10*s), zeros]; partition p
  holds column jb*128 + 127 - p -- the host un-reverses for free), so block
  jb's prefix factor matrix [col-part p, row-free m] is the uniform slice
  Ball[:, 896-jb*128 :][:W] (zeros at/beyond each column's diagonal give
  qt_j automatically).  Per (dd, jb):
    DVE: y = tensor_scalar(Ball_slice add c, min 1), then prefix
    tensor_tensor_scan with state = max(y*state, 0)  (== cumprod of
    clamp(y,0,1): a negative y zeroes the state, which then stays 0;
    fp32 state, bf16 downcast out).
    Act: tail rows in ONE op: exp(iota*ln(qt) + ln(R_j)) via the
    per-partition scale/bias of the activation instruction (iota built
    on-device by an add-scan; ln(qt) host-precomputed; ln(R_j) from a
    strided gather of the prefix ends + Ln).
  The scan result is already [col, row] = the transpose of the output, so a
  single 3D-AP DMA per doc-dir stores it column-major; the host transposes
  (memory regime: bf16 halves HBM store traffic).  The band ships as fp16:
  2-byte stream halves the DVE tensor_scalar input cost and the band load
  bytes, and fp16's 10 mantissa bits keep the B+c cancellation error small
  (global rel-err ~1.6e-3 << 2e-2; bf16 band would be 1.3e-2).

Sharding: pure data parallel, 8 doc-dirs per core (4 docs x fwd/bwd).
"""
import numpy as np

import concourse.bass as bass
import concourse.bacc as bacc
import concourse.tile as tile
from concourse import mybir
from concourse import bass_utils

P = 128
N = 1022          # columns j per doc-dir
ROWS = N - 1      # 1021 output rows (m axis)
NB = 8            # column blocks
ARRW = 1152       # band source width: max read index 127 + 1023 = 1150
BALLW = 1024      # band tile width (prefix slices all end at 1024)
IOTAW = 896       # longest tail is 1021 - 128 = 893

_NC_CACHE: dict = {}


def _act_loads_prefer_lnexp(self):
    """insert_act_table_loads variant that pins Ln/Exp to the one PWP set
    containing both ('natural_log_exp_and_others'), so alternating Ln/Exp
    activations share a single table load instead of thrashing (~1.3us per
    reload on the Act engine).  Set ids stay aligned with act_info.json."""
    import bass_rust as _bass_rust
    from concourse.hw_specs import get_activation_tables

    has_activation = any(
        isinstance(i, mybir.InstActivation)
        for b in self.main_func.blocks
        for i in b.instructions
    )
    if not has_activation:
        return
    tables = list(get_activation_tables(self.m.arch).items())
    ln_f = mybir.ActivationFunctionType.Ln
    exp_f = mybir.ActivationFunctionType.Exp
    both = [i for i, (nm, fs) in enumerate(tables) if ln_f in fs and exp_f in fs]
    if both:
        keep = both[0]
        tables = [
            (nm, fs if i == keep else {f for f in fs if f not in (ln_f, exp_f)})
            for i, (nm, fs) in enumerate(tables)
        ]
    _bass_rust.insert_act_table_loads(self, tables)


def build_nc(n_dd: int = 8):
    """Single-core Bass program processing n_dd doc-dirs, column-major out."""
    import types
    nc = bacc.Bacc("TRN2", target_bir_lowering=False, debug=False, num_devices=8)
    nc.insert_act_table_loads = types.MethodType(_act_loads_prefer_lnexp, nc)
    arr = nc.dram_tensor("arr", [n_dd, ARRW], mybir.dt.float16, kind="ExternalInput")
    cc = nc.dram_tensor("cc", [P, n_dd * 16], mybir.dt.float32, kind="ExternalInput")
    out = nc.dram_tensor(
        "out", [n_dd, NB, P, ROWS], mybir.dt.bfloat16, kind="ExternalOutput"
    )

    add_op = mybir.AluOpType.add
    min_op = mybir.AluOpType.min
    max_op = mybir.AluOpType.max
    mult_op = mybir.AluOpType.mult
    exp_f = mybir.ActivationFunctionType.Exp
    ln_f = mybir.ActivationFunctionType.Ln

    with tile.TileContext(nc) as tc:
        with (
            tc.tile_pool(name="consts", bufs=1) as consts,
            tc.tile_pool(name="band", bufs=4) as band_pool,
            tc.tile_pool(name="qpool", bufs=6) as qpool,
            tc.tile_pool(name="rpool", bufs=4) as rpool,
            tc.tile_pool(name="lpool", bufs=3) as lpool,
        ):
            zeros = consts.tile([P, 1], mybir.dt.float32)
            nc.vector.memset(zeros[:], 0.0)
            csb = consts.tile([P, n_dd * 16], mybir.dt.float32)
            nc.sync.dma_start(out=csb[:], in_=cc[:, :])
            ones = consts.tile([P, 1], mybir.dt.float32)
            nc.vector.memset(ones[:], 1.0)
            eps = consts.tile([P, 1], mybir.dt.float32)
            nc.vector.memset(eps[:], 1e-38)

            def zb(n):
                z = zeros[:, 0:1]
                return bass.AP(tensor=z.tensor, offset=z.offset,
                               ap=[z.ap[0], [0, n]])

            def ob(n):
                o = ones[:, 0:1]
                return bass.AP(tensor=o.tensor, offset=o.offset,
                               ap=[o.ap[0], [0, n]])

            # iota[p, m] = m + 1, built on-device: state = (1 add state) max 0
            iot = consts.tile([P, IOTAW], mybir.dt.float32)
            nc.vector.tensor_tensor_scan(
                out=iot[:], data0=ob(IOTAW), data1=zb(IOTAW), initial=0.0,
                op0=add_op, op1=max_op,
            )

            for dd in range(n_dd):
                Ball = band_pool.tile([P, BALLW], mybir.dt.float16, tag="Ball")
                band_src = bass.AP(
                    tensor=arr, offset=dd * ARRW, ap=[[1, P], [1, BALLW]]
                )
                # dd0's band rides the otherwise-idle scalar ring so it lands
                # in parallel with csb (sync ring) at startup
                beng = nc.scalar if dd == 0 else nc.sync
                beng.dma_start(out=Ball[:], in_=band_src)

                R = rpool.tile([P, NB * ROWS], mybir.dt.bfloat16, tag="R", name="R")
                lnr = lpool.tile([P, 8], mybir.dt.float32, tag="lnr", name="lnr")

                jb_order = (list(range(NB)) if dd < n_dd - 1
                            else [7, 0, 1, 2, 3, 4, 5, 6])
                for jb in jb_order:
                    off = 896 - jb * 128
                    W = min(128 * (jb + 1), ROWS)
                    jbR = jb * ROWS
                    Q = qpool.tile([P, ROWS], mybir.dt.float16, tag="Q", name="Q")
                    # y = min(B + c, 1); the lower clamp happens inside the
                    # scan: state = max(y*state, 0) == cumprod of clamp(y,0,1)
                    nc.vector.tensor_scalar(
                        out=Q[:, :W],
                        in0=Ball[:, off:off + W],
                        scalar1=csb[:, dd * 16 + jb:dd * 16 + jb + 1],
                        scalar2=1.0,
                        op0=add_op,
                        op1=min_op,
                    )
                    nc.vector.tensor_tensor_scan(
                        out=R[:, jbR:jbR + W],
                        data0=Q[:, :W],
                        data1=zb(W),
                        initial=1.0,
                        op0=mult_op,
                        op1=max_op,
                    )
                    if W < ROWS:
                        # tail rows: exp(iota*ln(qt) + ln(R_j)), ln read
                        # straight off the prefix end (bias guards ln(0));
                        # depends only on this block's scan -> fine pipeline
                        nc.scalar.activation(
                            out=lnr[:, jb:jb + 1],
                            in_=R[:, jbR + W - 1:jbR + W],
                            func=ln_f, bias=eps[:, 0:1], scale=1.0,
                        )
                        nc.scalar.activation(
                            out=R[:, jbR + W:jbR + ROWS],
                            in_=iot[:, 0:ROWS - W],
                            func=exp_f,
                            bias=lnr[:, jb:jb + 1],
                            scale=csb[:, dd * 16 + 8 + jb:dd * 16 + 8 + jb + 1],
                        )
                # last dd: final two blocks stored on separate rings so the
                # jb6 store overlaps the jb7 scan and the tail store is small
                chunks = [(0, 2), (2, 2), (4, 2), (6, 2)] if dd < n_dd - 1 else \
                         [(0, 2), (2, 2), (4, 2), (6, 1), (7, 1)]
                for h, (b0, nbh) in enumerate(chunks):
                    dst = bass.AP(
                        tensor=out,
                        offset=(dd * NB + b0) * P * ROWS,
                        ap=[[ROWS, P], [P * ROWS, nbh], [1, ROWS]],
                    )
                    deng = nc.sync if h % 2 == 0 else nc.scalar
                    deng.dma_start(
                        out=dst,
                        in_=R[:, b0 * ROWS:(b0 + nbh) * ROWS],
                    )
    nc.compile()
    return nc


def get_nc(n_dd: int = 8):
    if n_dd not in _NC_CACHE:
        _NC_CACHE[n_dd] = build_nc(n_dd)
    return _NC_CACHE[n_dd]


def make_core_inputs(docs_core: np.ndarray) -> dict:
    """docs_core: [n_docs, 1024] f32 -> in_map with arr/cc for n_docs*2 doc-dirs."""
    n_docs = docs_core.shape[0]
    n_dd = n_docs * 2
    arr = np.zeros((n_dd, ARRW), np.float16)
    cc = np.ones((P, n_dd * 16), np.float32)
    for dl in range(n_docs):
        s = docs_core[dl, 1:-1].astype(np.float32)  # 1022
        for t in range(2):
            v = s if t == 0 else s[::-1]
            dd = dl * 2 + t
            v10 = (np.float32(10.0) * v).astype(np.float32)
            arr[dd, 1:1 + N] = v10[::-1].astype(np.float16)
            cvals = np.ones(NB * P, np.float32)
            cvals[:N] = np.float32(1.0) - v10
            # partition p holds column jb*128 + 127 - p
            cpk = cvals.reshape(NB, P)[:, ::-1].T            # [P, NB]
            cc[:, dd * 16:dd * 16 + 8] = cpk
            qt = np.clip(cpk, 0.0, 1.0)
            cc[:, dd * 16 + 8:dd * 16 + 16] = np.log(
                np.maximum(qt, np.float32(1e-38))
            ).astype(np.float32)
    return {"arr": arr, "cc": cc}


def kernel(score: np.ndarray, score_idx: np.ndarray) -> np.ndarray:
    score = np.asarray(score, dtype=np.float32)
    score_idx = np.asarray(score_idx)
    docs = score[score_idx]  # [B, L] gather
    Bn, L = docs.shape       # 32, 1024
    n_cores = 8
    docs_per_core = Bn // n_cores  # 4

    in_maps = [
        make_core_inputs(docs[c * docs_per_core:(c + 1) * docs_per_core])
        for c in range(n_cores)
    ]
    nc = get_nc(docs_per_core * 2)
    res = bass_utils.run_bass_kernel_spmd(nc, in_maps, core_ids=list(range(n_cores)))
    full = np.empty((Bn, 2, ROWS, N), np.float32)
    for c in range(n_cores):
        o = np.asarray(res.results[c]["out"])  # [n_dd, NB, P, ROWS] bf16
        # partition p holds column jb*128 + 127 - p: un-reverse blocks
        o = o[:, :, ::-1, :].reshape(docs_per_core * 2, NB * P, ROWS)
        o = o.astype(np.float32)
        for dl in range(docs_per_core):
            for t in range(2):
                dd = dl * 2 + t
                full[c * docs_per_core + dl, t] = o[dd].T[:, :N]
    return full


# revision 42
# speedup vs baseline: 1.8604x; 1.8604x over previous
"""Trainium2 Bass kernel for nn_Gate_Net (Toeplitz + hard-sigmoid prob + cumprod gate).

Reference computation (per document row of 1024 scores):
  s = doc[1:-1]                      # n = 1022
  score_hat[r, j] = s[j-1-r] if j-1-r >= 0 else 0      # [1021, 1022]
  p[r, j] = clamp(10*(score_hat - s[j]) + 1, 0, 1)      # hard branch, res=0.1
  fwd = cumprod(p, axis=0); bwd = same with s reversed
  out = stack([fwd, bwd]) per doc -> full [32, 2, 1021, 1022] f32

Device algorithm (per doc-dir, column-major, transpose-free):
  Column j's factor sequence over rows m is
    q(j, m) = clamp(g[j-1-m] + c_j, 0, 1),  g[x] = 10*s[x] (0 outside [0,n)),
    c_j = 1 - 10*s[j].
  For m >= j the factor is the constant qt_j = clamp(c_j, 0, 1), so each
  column is geometric below its diagonal:
    out[m, j] = R_j * qt_j^(m-W+1)  for m >= W = 128*(jb+1) >= j+1,
  with R_j the cumprod at row W-1.  One band DMA per doc-dir materializes
  Ball[p, t] = arrR[p + t] (arrR = [0, reversed(10*s), zeros]; partition p
  holds column jb*128 + 127 - p -- the host un-reverses for free), so block
  jb's prefix factor matrix [col-part p, row-free m] is the uniform slice
  Ball[:, 896-jb*128 :][:W] (zeros at/beyond each column's diagonal give
  qt_j automatically).  Per (dd, jb):
    DVE: y = tensor_scalar(Ball_slice add c, min 1), then prefix
    tensor_tensor_scan with state = max(y*state, 0)  (== cumprod of
    clamp(y,0,1): a negative y zeroes the state, which then stays 0;
    fp32 state, bf16 downcast out).
    Act: tail rows in ONE op: exp(iota*ln(qt) + ln(R_j)) via the
    per-partition scale/bias of the activation instruction (iota built
    on-device by an add-scan; ln(qt) host-precomputed; ln(R_j) from a
    strided gather of the prefix ends + Ln).
  The scan result is already [col, row] = the transpose of the output, so a
  single 3D-AP DMA per doc-dir stores it column-major; the host transposes
  (memory regime: bf16 halves HBM store traffic).  The band ships as fp16:
  2-byte stream halves the DVE tensor_scalar input cost and the band load
  bytes, and fp16's 10 mantissa bits keep the B+c cancellation error small
  (global rel-err ~1.6e-3 << 2e-2; bf16 band would be 1.3e-2).

Sharding: pure data parallel, 8 doc-dirs per core (4 docs x fwd/bwd).
"""
import numpy as np

import concourse.bass as bass
import concourse.bacc as bacc
import concourse.tile as tile
from concourse import mybir
from concourse import bass_utils

P = 128
N = 1022          # columns j per doc-dir
ROWS = N - 1      # 1021 output rows (m axis)
NB = 8            # column blocks
ARRW = 1152       # band source width: max read index 127 + 1023 = 1150
BALLW = 1024      # band tile width (prefix slices all end at 1024)
IOTAW = 896       # longest tail is 1021 - 128 = 893

_NC_CACHE: dict = {}


def _act_loads_prefer_lnexp(self):
    """insert_act_table_loads variant that pins Ln/Exp to the one PWP set
    containing both ('natural_log_exp_and_others'), so alternating Ln/Exp
    activations share a single table load instead of thrashing (~1.3us per
    reload on the Act engine).  Set ids stay aligned with act_info.json.
    Falls back to the stock pass if the table catalog differs."""
    try:
        import bass_rust as _bass_rust
        from concourse.hw_specs import get_activation_tables

        has_activation = any(
            isinstance(i, mybir.InstActivation)
            for b in self.main_func.blocks
            for i in b.instructions
        )
        if not has_activation:
            return
        tables = list(get_activation_tables(self.m.arch).items())
        ln_f = mybir.ActivationFunctionType.Ln
        exp_f = mybir.ActivationFunctionType.Exp
        both = [i for i, (nm, fs) in enumerate(tables)
                if ln_f in fs and exp_f in fs]
        if both:
            keep = both[0]
            tables = [
                (nm, fs if i == keep
                 else {f for f in fs if f not in (ln_f, exp_f)})
                for i, (nm, fs) in enumerate(tables)
            ]
        _bass_rust.insert_act_table_loads(self, tables)
    except Exception:
        bacc.Bacc.insert_act_table_loads(self)


def build_nc(n_dd: int = 8):
    """Single-core Bass program processing n_dd doc-dirs, column-major out."""
    import types
    nc = bacc.Bacc("TRN2", target_bir_lowering=False, debug=False, num_devices=8)
    nc.insert_act_table_loads = types.MethodType(_act_loads_prefer_lnexp, nc)
    arr = nc.dram_tensor("arr", [n_dd, ARRW], mybir.dt.float16, kind="ExternalInput")
    cc = nc.dram_tensor("cc", [P, n_dd * 16], mybir.dt.float32, kind="ExternalInput")
    out = nc.dram_tensor(
        "out", [n_dd, NB, P, ROWS], mybir.dt.bfloat16, kind="ExternalOutput"
    )

    add_op = mybir.AluOpType.add
    min_op = mybir.AluOpType.min
    max_op = mybir.AluOpType.max
    mult_op = mybir.AluOpType.mult
    exp_f = mybir.ActivationFunctionType.Exp
    ln_f = mybir.ActivationFunctionType.Ln

    with tile.TileContext(nc) as tc:
        with (
            tc.tile_pool(name="consts", bufs=1) as consts,
            tc.tile_pool(name="band", bufs=4) as band_pool,
            tc.tile_pool(name="qpool", bufs=6) as qpool,
            tc.tile_pool(name="rpool", bufs=4) as rpool,
            tc.tile_pool(name="lpool", bufs=3) as lpool,
        ):
            zeros = consts.tile([P, 1], mybir.dt.float32)
            nc.vector.memset(zeros[:], 0.0)
            csb = consts.tile([P, n_dd * 16], mybir.dt.float32)
            nc.sync.dma_start(out=csb[:], in_=cc[:, :])
            ones = consts.tile([P, 1], mybir.dt.float32)
            nc.vector.memset(ones[:], 1.0)
            eps = consts.tile([P, 1], mybir.dt.float32)
            nc.vector.memset(eps[:], 1e-38)

            def zb(n):
                z = zeros[:, 0:1]
                return bass.AP(tensor=z.tensor, offset=z.offset,
                               ap=[z.ap[0], [0, n]])

            def ob(n):
                o = ones[:, 0:1]
                return bass.AP(tensor=o.tensor, offset=o.offset,
                               ap=[o.ap[0], [0, n]])

            # iota[p, m] = m + 1, built on-device: state = (1 add state) max 0
            iot = consts.tile([P, IOTAW], mybir.dt.float32)
            nc.vector.tensor_tensor_scan(
                out=iot[:], data0=ob(IOTAW), data1=zb(IOTAW), initial=0.0,
                op0=add_op, op1=max_op,
            )

            for dd in range(n_dd):
                Ball = band_pool.tile([P, BALLW], mybir.dt.float16, tag="Ball")
                band_src = bass.AP(
                    tensor=arr, offset=dd * ARRW, ap=[[1, P], [1, BALLW]]
                )
                # dd0's band rides the otherwise-idle scalar ring so it lands
                # in parallel with csb (sync ring) at startup
                beng = nc.scalar if dd == 0 else nc.sync
                beng.dma_start(out=Ball[:], in_=band_src)

                R = rpool.tile([P, NB * ROWS], mybir.dt.bfloat16, tag="R", name="R")
                lnr = lpool.tile([P, 8], mybir.dt.float32, tag="lnr", name="lnr")

                jb_order = (list(range(NB)) if dd < n_dd - 1
                            else [7, 0, 1, 2, 3, 4, 5, 6])
                for jb in jb_order:
                    off = 896 - jb * 128
                    W = min(128 * (jb + 1), ROWS)
                    jbR = jb * ROWS
                    Q = qpool.tile([P, ROWS], mybir.dt.float16, tag="Q", name="Q")
                    # y = min(B + c, 1); the lower clamp happens inside the
                    # scan: state = max(y*state, 0) == cumprod of clamp(y,0,1)
                    nc.vector.tensor_scalar(
                        out=Q[:, :W],
                        in0=Ball[:, off:off + W],
                        scalar1=csb[:, dd * 16 + jb:dd * 16 + jb + 1],
                        scalar2=1.0,
                        op0=add_op,
                        op1=min_op,
                    )
                    nc.vector.tensor_tensor_scan(
                        out=R[:, jbR:jbR + W],
                        data0=Q[:, :W],
                        data1=zb(W),
                        initial=1.0,
                        op0=mult_op,
                        op1=max_op,
                    )
                    if W < ROWS:
                        # tail rows: exp(iota*ln(qt) + ln(R_j)), ln read
                        # straight off the prefix end (bias guards ln(0));
                        # depends only on this block's scan -> fine pipeline
                        nc.scalar.activation(
                            out=lnr[:, jb:jb + 1],
                            in_=R[:, jbR + W - 1:jbR + W],
                            func=ln_f, bias=eps[:, 0:1], scale=1.0,
                        )
                        nc.scalar.activation(
                            out=R[:, jbR + W:jbR + ROWS],
                            in_=iot[:, 0:ROWS - W],
                            func=exp_f,
                            bias=lnr[:, jb:jb + 1],
                            scale=csb[:, dd * 16 + 8 + jb:dd * 16 + 8 + jb + 1],
                        )
                # last dd: final two blocks stored on separate rings so the
                # jb6 store overlaps the jb7 scan and the tail store is small
                chunks = [(0, 2), (2, 2), (4, 2), (6, 2)] if dd < n_dd - 1 else \
                         [(0, 2), (2, 2), (4, 2), (6, 1), (7, 1)]
                for h, (b0, nbh) in enumerate(chunks):
                    dst = bass.AP(
                        tensor=out,
                        offset=(dd * NB + b0) * P * ROWS,
                        ap=[[ROWS, P], [P * ROWS, nbh], [1, ROWS]],
                    )
                    deng = nc.sync if h % 2 == 0 else nc.scalar
                    deng.dma_start(
                        out=dst,
                        in_=R[:, b0 * ROWS:(b0 + nbh) * ROWS],
                    )
    nc.compile()
    return nc


def get_nc(n_dd: int = 8):
    if n_dd not in _NC_CACHE:
        _NC_CACHE[n_dd] = build_nc(n_dd)
    return _NC_CACHE[n_dd]


def make_core_inputs(docs_core: np.ndarray) -> dict:
    """docs_core: [n_docs, 1024] f32 -> in_map with arr/cc for n_docs*2 doc-dirs."""
    n_docs = docs_core.shape[0]
    n_dd = n_docs * 2
    arr = np.zeros((n_dd, ARRW), np.float16)
    cc = np.ones((P, n_dd * 16), np.float32)
    for dl in range(n_docs):
        s = docs_core[dl, 1:-1].astype(np.float32)  # 1022
        for t in range(2):
            v = s if t == 0 else s[::-1]
            dd = dl * 2 + t
            v10 = (np.float32(10.0) * v).astype(np.float32)
            arr[dd, 1:1 + N] = v10[::-1].astype(np.float16)
            cvals = np.ones(NB * P, np.float32)
            cvals[:N] = np.float32(1.0) - v10
            # partition p holds column jb*128 + 127 - p
            cpk = cvals.reshape(NB, P)[:, ::-1].T            # [P, NB]
            cc[:, dd * 16:dd * 16 + 8] = cpk
            qt = np.clip(cpk, 0.0, 1.0)
            cc[:, dd * 16 + 8:dd * 16 + 16] = np.log(
                np.maximum(qt, np.float32(1e-38))
            ).astype(np.float32)
    return {"arr": arr, "cc": cc}


def kernel(score: np.ndarray, score_idx: np.ndarray) -> np.ndarray:
    score = np.asarray(score, dtype=np.float32)
    score_idx = np.asarray(score_idx)
    docs = score[score_idx]  # [B, L] gather
    Bn, L = docs.shape       # 32, 1024
    n_cores = 8
    docs_per_core = Bn // n_cores  # 4

    in_maps = [
        make_core_inputs(docs[c * docs_per_core:(c + 1) * docs_per_core])
        for c in range(n_cores)
    ]
    nc = get_nc(docs_per_core * 2)
    res = bass_utils.run_bass_kernel_spmd(nc, in_maps, core_ids=list(range(n_cores)))
    full = np.empty((Bn, 2, ROWS, N), np.float32)
    for c in range(n_cores):
        o = np.asarray(res.results[c]["out"])  # [n_dd, NB, P, ROWS] bf16
        # partition p holds column jb*128 + 127 - p: un-reverse blocks
        o = o[:, :, ::-1, :].reshape(docs_per_core * 2, NB * P, ROWS)
        o = o.astype(np.float32)
        for dl in range(docs_per_core):
            for t in range(2):
                dd = dl * 2 + t
                full[c * docs_per_core + dl, t] = o[dd].T[:, :N]
    return full
